# revision 18
# baseline (speedup 1.0000x reference)
"""Trainium2 Bass kernel for nn_CrossAttentionDecoder (ragged cross-attention
transformer block + pixel-unshuffle decode heads).

Strategy: data-parallel over batch b=16 -> 2 batches per NeuronCore x 8 cores.
All activations are kept feature-major on device ((C partitions, tokens free))
so no transposes are ever needed:

  - LN1 is folded on the host into the Q projection (wq1 = diag(g1) @ wq,
    cq = colsum(wq1), vq = ln1_b @ wq + bq).  Per-token mean enters the
    projection as a rank-1 matmul term; the 1/std score scale and the bias
    term enter the score matmul via a K=1 augmentation against s1[q], then a
    single fused (psum + klog)*r1 pass + Exp produce masked exp-scores.
  - The ragged causal mask is a key-PREFIX per (segment, batch) because tk is
    sorted, so it is a per-partition additive -1e9 column (klog); invalid
    (padded) query segments get scale 0 -> exp(0)=1 -> exactly the uniform
    softmax the reference produces for fully-masked rows.
  - Softmax denominators are computed with ones-matmuls (partition reduce on
    PE) and applied to the attention output during PSUM evacuation.
  - LN2 is computed on device via ones-matmul moments; its scale/bias are
    folded into w1 (host) the same way LN1 was.
  - The pixel-unshuffle + final reshapes of the decode heads are pure data
    movement and happen on the host after gathering per-core outputs.
"""

import sys

if "/opt/trn_rl_repo" not in sys.path:
    sys.path.insert(0, "/opt/trn_rl_repo")

import numpy as np

S, B, C, H, DH = 16, 16, 256, 4, 64
HW_TABLE, HW_HAND = 256, 36
HW = HW_TABLE + HW_HAND + 1  # 293
LK = 128
LQ = S * HW  # 4688
NCORES = 8
NB = B // NCORES  # 2 batches per core
WTILE = 512
EPS = 1e-5
F32 = np.float32


def _qtiles():
    """[(q0, w, [(seg, rel_lo, rel_hi, hw_lo), ...]), ...] covering [0, LQ)."""
    tiles = []
    q0 = 0
    while q0 < LQ:
        w = min(WTILE, LQ - q0)
        chunks = []
        s = q0 // HW
        while s * HW < q0 + w:
            lo = max(q0, s * HW)
            hi = min(q0 + w, (s + 1) * HW)
            chunks.append((s, lo - q0, hi - q0, lo - s * HW))
            s += 1
        tiles.append((q0, w, chunks))
        q0 += w
    return tiles


# --------------------------------------------------------------------------
# device program
# --------------------------------------------------------------------------

_PROGRAM_CACHE = {}
_LAST_RES = None


def _build_program(vq_any, bv_any):
    import concourse.tile as tile
    from concourse import bacc, mybir

    f32 = mybir.dt.float32
    AF = mybir.ActivationFunctionType
    OP = mybir.AluOpType

    nc = bacc.Bacc("TRN2", debug=False)

    def din(name, *shape):
        return nc.dram_tensor(name, list(shape), f32, kind="ExternalInput").ap()

    def dout(name, *shape):
        return nc.dram_tensor(name, list(shape), f32, kind="ExternalOutput").ap()

    # replicated weights (host-transformed)
    d_spatialT = din("spatialT", 2, 128, HW)
    d_wq1 = din("wq1", 2, 2, 128, 128)      # [cin_blk, cout_blk]
    d_wk = din("wk", 2, 2, 128, 128)
    d_wv = din("wv", 2, 128, C)             # [cin_blk] rhs layout
    d_wo = din("wo", 2, 2, 128, 128)
    d_w1p = din("w1p", 2, 8, 128, 128)      # [cin_blk, j_blk]
    d_w2 = din("w2", 8, 2, 128, 128)        # [j_blk, cout_blk]
    d_tablew = din("tablew", 2, 128, 32)
    d_handw = din("handw", 2, 128, 32)
    d_catw = din("catw", 2, 128, 100)
    d_cq = din("cq", 1, C)
    d_bvr = din("bvr", 1, C)
    d_vqc = din("vqc", 128, 2)
    d_bkc = din("bkc", 128, 2)
    d_boc = din("boc", 128, 2)
    d_b2c = din("b2c", 128, 2)
    d_v1c = din("v1c", 128, 8)
    d_tbc = din("tbc", 32, 1)
    d_hbc = din("hbc", 32, 1)
    d_cbc = din("cbc", 100, 1)
    # per-batch data
    d_xkT = din("xkT", NB, 2, 128, 128)     # [b, cin_blk, c, k]
    d_xtc = din("xtc", NB, 128, S, 2)       # [b, c_in_blk_part, s, blk]
    d_rows = din("rows", NB, 2, LQ)         # [b, {negm1, s1}, q]
    d_r1b = din("r1b", NB, 128, LQ)         # r1/8 row replicated to 128 parts
    d_klog = din("klog", NB, 128, S)        # (kmask-1)*1e9  [b, k, s]

    d_txs = dout("txs", NB, S, 32, HW_TABLE)
    d_hxs = dout("hxs", NB, S, 32, HW_HAND)
    d_lgs = dout("lgs", NB, S, 100)

    qtiles = _qtiles()

    with tile.TileContext(nc) as tc:
        with (
            tc.tile_pool(name="consts", bufs=1) as cp,
            tc.tile_pool(name="batch", bufs=1) as bp,
            tc.tile_pool(name="big", bufs=1) as bigp,
            tc.tile_pool(name="work", bufs=2) as wp,
            tc.tile_pool(name="works", bufs=2) as wps,
            tc.tile_pool(name="pt", bufs=4) as ptp,
            tc.tile_pool(name="hpool", bufs=1) as hp,
            tc.tile_pool(name="rowp", bufs=4) as rp,
            tc.tile_pool(name="psbig", bufs=3, space="PSUM") as psb,
            tc.tile_pool(name="psav", bufs=3, space="PSUM") as psa,
            tc.tile_pool(name="psrow", bufs=2, space="PSUM") as psr,
        ):
            # ---- load constants -------------------------------------------------
            def ctile(shape, src, tag):
                t = cp.tile(shape, f32, tag=tag)
                nc.sync.dma_start(out=t, in_=src)
                return t

            spatialT = cp.tile([128, 2, HW], f32)
            for blk in range(2):
                nc.sync.dma_start(out=spatialT[:, blk, :], in_=d_spatialT[blk])
            wq1 = cp.tile([128, 2, 2, 128], f32)
            wk = cp.tile([128, 2, 2, 128], f32)
            wo = cp.tile([128, 2, 2, 128], f32)
            for cb in range(2):
                for ob in range(2):
                    nc.sync.dma_start(out=wq1[:, cb, ob, :], in_=d_wq1[cb, ob])
                    nc.sync.dma_start(out=wk[:, cb, ob, :], in_=d_wk[cb, ob])
                    nc.sync.dma_start(out=wo[:, cb, ob, :], in_=d_wo[cb, ob])
            wv = cp.tile([128, 2, C], f32)
            for cb in range(2):
                nc.sync.dma_start(out=wv[:, cb, :], in_=d_wv[cb])
            w1p = cp.tile([128, 2, 8, 128], f32)
            for cb in range(2):
                for jb in range(8):
                    nc.sync.dma_start(out=w1p[:, cb, jb, :], in_=d_w1p[cb, jb])
            w2 = cp.tile([128, 8, 2, 128], f32)
            for jb in range(8):
                for ob in range(2):
                    nc.sync.dma_start(out=w2[:, jb, ob, :], in_=d_w2[jb, ob])
            tablew = cp.tile([128, 2, 32], f32)
            handw = cp.tile([128, 2, 32], f32)
            catw = cp.tile([128, 2, 100], f32)
            for cb in range(2):
                nc.sync.dma_start(out=tablew[:, cb, :], in_=d_tablew[cb])
                nc.sync.dma_start(out=handw[:, cb, :], in_=d_handw[cb])
                nc.sync.dma_start(out=catw[:, cb, :], in_=d_catw[cb])
            cq = ctile([1, C], d_cq, "cq")
            bvr = ctile([1, C], d_bvr, "bvr")
            vqc = ctile([128, 2], d_vqc, "vqc")
            bkc = ctile([128, 2], d_bkc, "bkc")
            boc = ctile([128, 2], d_boc, "boc")
            b2c = ctile([128, 2], d_b2c, "b2c")
            v1c = ctile([128, 8], d_v1c, "v1c")
            tbc = ctile([32, 1], d_tbc, "tbc")
            hbc = ctile([32, 1], d_hbc, "hbc")
            cbc = ctile([100, 1], d_cbc, "cbc")
            ones_col = cp.tile([128, 1], f32)
            nc.vector.memset(ones_col, 1.0)
            ones_m = cp.tile([1, 128], f32)
            nc.vector.memset(ones_m, 1.0)
            eps_t = cp.tile([1, 1], f32)
            nc.vector.memset(eps_t, EPS)

            for b in range(NB):
                # ---- per-batch loads -------------------------------------------
                xkT = bp.tile([128, 2, 128], f32, tag="xkT")
                for cb in range(2):
                    nc.sync.dma_start(out=xkT[:, cb, :], in_=d_xkT[b, cb])
                xtc = bp.tile([128, S, 2], f32, tag="xtc")
                nc.sync.dma_start(out=xtc, in_=d_xtc[b])
                klog = bp.tile([128, S], f32, tag="klog")
                nc.sync.dma_start(out=klog, in_=d_klog[b])

                # ---- K^T, V, vqK ----------------------------------------------
                KT = bp.tile([128, 2, 128], f32, tag="KT")
                for ob in range(2):
                    ps = psb.tile([128, 128], f32, tag="big")
                    nc.tensor.matmul(ps, wk[:, 0, ob, :], xkT[:, 0, :], start=True, stop=False)
                    nc.tensor.matmul(ps, wk[:, 1, ob, :], xkT[:, 1, :], start=False, stop=True)
                    nc.vector.tensor_scalar_add(KT[:, ob, :], ps, bkc[:, ob : ob + 1])
                V = bp.tile([128, C], f32, tag="V")
                psv = psb.tile([128, C], f32, tag="big")
                nc.tensor.matmul(psv, xkT[:, 0, :], wv[:, 0, :], start=True, stop=False)
                nc.tensor.matmul(psv, xkT[:, 1, :], wv[:, 1, :], start=False, stop=not bv_any)
                if bv_any:
                    nc.tensor.matmul(psv, ones_m, bvr, start=False, stop=True)
                nc.scalar.copy(out=V, in_=psv)
                if vq_any:
                    vqK = bp.tile([1, 128], f32, tag="vqK")
                    psq = psr.tile([1, 128], f32, tag="row")
                    nc.tensor.matmul(psq, vqc[:, 0:1], KT[:, 0, :], start=True, stop=False)
                    nc.tensor.matmul(psq, vqc[:, 1:2], KT[:, 1, :], start=False, stop=True)
                    nc.scalar.copy(out=vqK, in_=psq)

                x2r = bigp.tile([128, 2, LQ], f32, tag="x2")

                for (q0, w, chunks) in qtiles:
                    qr = slice(q0, q0 + w)
                    # ---- xq build ---------------------------------------------
                    xq = wp.tile([128, 2, w], f32, tag="xq")
                    for blk in range(2):
                        for (s, lo, hi, hw0) in chunks:
                            nc.vector.tensor_scalar_add(
                                xq[:, blk, lo:hi],
                                spatialT[:, blk, hw0 : hw0 + (hi - lo)],
                                xtc[:, s : s + 1, blk],
                            )
                    rows_t = wps.tile([1, 2, w], f32, tag="rows")
                    nc.sync.dma_start(out=rows_t, in_=d_rows[b : b + 1, :, qr])
                    # ---- A = wq1^T xq - cq x m1 -------------------------------
                    A = wp.tile([128, 2, w], f32, tag="A")
                    for ob in range(2):
                        ps = psb.tile([128, w], f32, tag="big")
                        nc.tensor.matmul(ps, wq1[:, 0, ob, :], xq[:, 0, :], start=True, stop=False)
                        nc.tensor.matmul(ps, wq1[:, 1, ob, :], xq[:, 1, :], start=False, stop=False)
                        nc.tensor.matmul(
                            ps, cq[:, 128 * ob : 128 * ob + 128], rows_t[0:1, 0, :],
                            start=False, stop=True,
                        )
                        nc.scalar.copy(out=A[:, ob, :], in_=ps)
                    # ---- r1/8 tile --------------------------------------------
                    r1t = wps.tile([128, w], f32, tag="r1t")
                    nc.sync.dma_start(out=r1t, in_=d_r1b[b, :, qr])
                    # ---- attention per head -----------------------------------
                    a_sb = wp.tile([128, 2, w], f32, tag="a")
                    for h in range(H):
                        hb, hh = h // 2, (h % 2) * 64
                        ps_s = psb.tile([128, w], f32, tag="big")
                        nc.tensor.matmul(
                            ps_s, KT[hh : hh + 64, hb, :], A[hh : hh + 64, hb, :],
                            start=True, stop=not vq_any,
                        )
                        if vq_any:
                            nc.tensor.matmul(ps_s, vqK, rows_t[0:1, 1, :], start=False, stop=True)
                        Pt = ptp.tile([128, w], f32, tag="pt")
                        for (s, lo, hi, _hw0) in chunks:
                            nc.vector.scalar_tensor_tensor(
                                out=Pt[:, lo:hi], in0=ps_s[:, lo:hi],
                                scalar=klog[:, s : s + 1], in1=r1t[:, lo:hi],
                                op0=OP.add, op1=OP.mult,
                            )
                        nc.scalar.activation(out=Pt, in_=Pt, func=AF.Exp)
                        ps_R = psr.tile([1, w], f32, tag="row")
                        nc.tensor.matmul(ps_R, ones_col, Pt, start=True, stop=True)
                        rinv = rp.tile([1, w], f32, tag="rinv")
                        nc.vector.reciprocal(out=rinv, in_=ps_R)
                        ps_rb = psa.tile([64, w], f32, tag="av")
                        nc.tensor.matmul(ps_rb, ones_m[:, 0:64], rinv, start=True, stop=True)
                        rinvb = wps.tile([64, w], f32, tag="rinvb")
                        nc.scalar.copy(out=rinvb, in_=ps_rb)
                        ps_av = psa.tile([64, w], f32, tag="av")
                        nc.tensor.matmul(ps_av, V[:, 64 * h : 64 * h + 64], Pt, start=True, stop=True)
                        nc.vector.tensor_tensor(
                            out=a_sb[hh : hh + 64, hb, :], in0=ps_av, in1=rinvb, op=OP.mult
                        )
                    # ---- x1 = xq + a @ wo + bo --------------------------------
                    x1 = wp.tile([128, 2, w], f32, tag="x1")
                    for ob in range(2):
                        ps = psb.tile([128, w], f32, tag="big")
                        nc.tensor.matmul(ps, wo[:, 0, ob, :], a_sb[:, 0, :], start=True, stop=False)
                        nc.tensor.matmul(ps, wo[:, 1, ob, :], a_sb[:, 1, :], start=False, stop=True)
                        nc.vector.scalar_tensor_tensor(
                            out=x1[:, ob, :], in0=ps, scalar=boc[:, ob : ob + 1],
                            in1=xq[:, ob, :], op0=OP.add, op1=OP.add,
                        )
                    # ---- LN2 moments ------------------------------------------
                    sq = wp.tile([128, 2, w], f32, tag="sq")
                    for ob in range(2):
                        nc.scalar.square(out=sq[:, ob, :], in_=x1[:, ob, :])
                    ps_sum = psr.tile([1, w], f32, tag="row")
                    nc.tensor.matmul(ps_sum, ones_col, x1[:, 0, :], start=True, stop=False)
                    nc.tensor.matmul(ps_sum, ones_col, x1[:, 1, :], start=False, stop=True)
                    ps_sq = psr.tile([1, w], f32, tag="row")
                    nc.tensor.matmul(ps_sq, ones_col, sq[:, 0, :], start=True, stop=False)
                    nc.tensor.matmul(ps_sq, ones_col, sq[:, 1, :], start=False, stop=True)
                    m2 = rp.tile([1, w], f32, tag="rowtmp")
                    nc.vector.tensor_scalar_mul(m2, ps_sum, 1.0 / C)
                    msq = rp.tile([1, w], f32, tag="rowtmp")
                    nc.vector.tensor_mul(msq, m2, m2)
                    var = rp.tile([1, w], f32, tag="rowtmp")
                    nc.vector.scalar_tensor_tensor(
                        out=var, in0=ps_sq, scalar=1.0 / C, in1=msq,
                        op0=OP.mult, op1=OP.subtract,
                    )
                    std = rp.tile([1, w], f32, tag="rowtmp")
                    nc.scalar.activation(out=std, in_=var, func=AF.Sqrt, bias=eps_t)
                    r2 = rp.tile([1, w], f32, tag="rowtmp")
                    nc.vector.reciprocal(out=r2, in_=std)
                    ps_m2b = psb.tile([128, w], f32, tag="big")
                    nc.tensor.matmul(ps_m2b, ones_m, m2, start=True, stop=True)
                    m2b = wps.tile([128, w], f32, tag="m2b")
                    nc.scalar.copy(out=m2b, in_=ps_m2b)
                    ps_r2b = psb.tile([128, w], f32, tag="big")
                    nc.tensor.matmul(ps_r2b, ones_m, r2, start=True, stop=True)
                    r2b = wps.tile([128, w], f32, tag="r2b")
                    nc.scalar.copy(out=r2b, in_=ps_r2b)
                    # ---- x~ = (x1 - m2) * r2 ----------------------------------
                    xt_ = wp.tile([128, 2, w], f32, tag="xt_")
                    for ob in range(2):
                        nc.vector.tensor_sub(xt_[:, ob, :], x1[:, ob, :], m2b)
                        nc.vector.tensor_mul(xt_[:, ob, :], xt_[:, ob, :], r2b)
                    # ---- FFN --------------------------------------------------
                    hbuf = hp.tile([128, 8, w], f32, tag="h")
                    for jb in range(8):
                        ps = psb.tile([128, w], f32, tag="big")
                        nc.tensor.matmul(ps, w1p[:, 0, jb, :], xt_[:, 0, :], start=True, stop=False)
                        nc.tensor.matmul(ps, w1p[:, 1, jb, :], xt_[:, 1, :], start=False, stop=True)
                        nc.scalar.activation(
                            out=hbuf[:, jb, :], in_=ps, func=AF.Gelu_apprx_tanh,
                            bias=v1c[:, jb : jb + 1],
                        )
                    for ob in range(2):
                        ps = psb.tile([128, w], f32, tag="big")
                        for jb in range(8):
                            nc.tensor.matmul(
                                ps, w2[:, jb, ob, :], hbuf[:, jb, :],
                                start=(jb == 0), stop=(jb == 7),
                            )
                        nc.vector.scalar_tensor_tensor(
                            out=x2r[:, ob, qr], in0=ps, scalar=b2c[:, ob : ob + 1],
                            in1=x1[:, ob, :], op0=OP.add, op1=OP.add,
                        )

                # ---- decode heads ---------------------------------------------
                for s in range(S):
                    base = s * HW
                    ps_t = psr.tile([32, HW_TABLE], f32, tag="row")
                    nc.tensor.matmul(ps_t, tablew[:, 0, :], x2r[:, 0, base : base + HW_TABLE], start=True, stop=False)
                    nc.tensor.matmul(ps_t, tablew[:, 1, :], x2r[:, 1, base : base + HW_TABLE], start=False, stop=True)
                    tx = rp.tile([32, HW_TABLE], f32, tag="tx")
                    nc.vector.tensor_scalar_add(tx, ps_t, tbc)
                    nc.sync.dma_start(out=d_txs[b, s], in_=tx)
                    hlo = base + HW_TABLE
                    ps_h = psr.tile([32, HW_HAND], f32, tag="row")
                    nc.tensor.matmul(ps_h, handw[:, 0, :], x2r[:, 0, hlo : hlo + HW_HAND], start=True, stop=False)
                    nc.tensor.matmul(ps_h, handw[:, 1, :], x2r[:, 1, hlo : hlo + HW_HAND], start=False, stop=True)
                    hx = rp.tile([32, HW_HAND], f32, tag="hx")
                    nc.vector.tensor_scalar_add(hx, ps_h, hbc)
                    nc.sync.dma_start(out=d_hxs[b, s], in_=hx)
                    glo = base + HW - 1
                    ps_l = psr.tile([100, 1], f32, tag="row")
                    nc.tensor.matmul(ps_l, catw[:, 0, :], x2r[:, 0, glo : glo + 1], start=True, stop=False)
                    nc.tensor.matmul(ps_l, catw[:, 1, :], x2r[:, 1, glo : glo + 1], start=False, stop=True)
                    lg = rp.tile([100, 1], f32, tag="lg")
                    nc.vector.tensor_scalar_add(lg, ps_l, cbc)
                    nc.sync.dma_start(out=d_lgs[b, s], in_=lg)

    nc.finalize()
    return nc


def _get_program(vq_any, bv_any):
    key = (bool(vq_any), bool(bv_any))
    if key not in _PROGRAM_CACHE:
        _PROGRAM_CACHE[key] = _build_program(*key)
    return _PROGRAM_CACHE[key]


# --------------------------------------------------------------------------
# host side
# --------------------------------------------------------------------------

def _blk2(w):  # (256, N) -> (2, 128, N)
    return np.ascontiguousarray(w.reshape(2, 128, -1))


def _blocks(w, nin, nout):  # (nin*128, nout*128) -> (nin, nout, 128, 128)
    return np.ascontiguousarray(
        w.reshape(nin, 128, nout, 128).transpose(0, 2, 1, 3)
    )


def _cols(v):  # (n*128,) -> (128, n) partition-major columns
    return np.ascontiguousarray(v.reshape(-1, 128).T)


def kernel(**inputs):
    f = lambda k: np.ascontiguousarray(np.asarray(inputs[k], F32))
    tq = np.asarray(inputs["tq"])
    pad_q = np.asarray(inputs["pad_q"])
    tk = np.asarray(inputs["tk"])
    pad_k = np.asarray(inputs["pad_k"])
    xk = f("xk")
    spatial = f("spatial_enc")
    temporal = f("temporal_enc")
    g1, bl1 = f("ln1_g"), f("ln1_b")
    g2, bl2 = f("ln2_g"), f("ln2_b")
    wq, bq = f("wq"), f("bq")
    wk_, bk = f("wk"), f("bk")
    wv_, bv = f("wv"), f("bv")
    wo_, bo = f("wo"), f("bo")
    w1, b1 = f("w1"), f("b1")
    w2_, b2 = f("w2"), f("b2")

    wq1 = wq * g1[:, None]
    cq = wq1.sum(0)
    vq = (bl1 @ wq + bq).astype(F32)
    w1p = w1 * g2[:, None]
    v1 = (bl2 @ w1 + b1).astype(F32)
    catW = np.concatenate([f("mode_w"), f("shape_w"), f("color_w")], 1)
    catB = np.concatenate([f("mode_b"), f("shape_b"), f("color_b")])
    vq_any = bool(np.any(vq))
    bv_any = bool(np.any(bv))

    # per-token LN1 stats (host): xq = temporal[tq] + spatial
    xq = temporal[tq][:, None, :, :] + spatial[None, :, None, :]  # (S,HW,B,C)
    m1 = xq.mean(-1)
    r1 = 1.0 / np.sqrt(xq.var(-1) + EPS)
    s1 = 1.0 / r1
    r1o8 = (r1 / 8.0).astype(F32)
    for b in range(B):
        r1o8[pad_q[b] :, :, b] = 0.0
    klog = np.zeros((B, LK, S), F32)
    for b in range(B):
        for s in range(S):
            if s < pad_q[b]:
                n1 = min(int((tk[:, b] <= tq[s, b]).sum()), int(pad_k[b]))
                klog[b, n1:, s] = -1e9

    shared = {
        "spatialT": np.ascontiguousarray(spatial.T.reshape(2, 128, HW)),
        "wq1": _blocks(wq1, 2, 2),
        "wk": _blocks(wk_, 2, 2),
        "wv": _blk2(wv_),
        "wo": _blocks(wo_, 2, 2),
        "w1p": _blocks(w1p, 2, 8),
        "w2": _blocks(w2_, 8, 2),
        "tablew": _blk2(f("table_w")),
        "handw": _blk2(f("hand_w")),
        "catw": _blk2(catW),
        "cq": np.ascontiguousarray(cq.reshape(1, C)),
        "bvr": np.ascontiguousarray(bv.reshape(1, C)),
        "vqc": _cols(vq),
        "bkc": _cols(bk),
        "boc": _cols(bo),
        "b2c": _cols(b2),
        "v1c": _cols(v1),
        "tbc": np.ascontiguousarray(f("table_b").reshape(32, 1)),
        "hbc": np.ascontiguousarray(f("hand_b").reshape(32, 1)),
        "cbc": np.ascontiguousarray(catB.reshape(100, 1)),
    }

    in_maps = []
    for c in range(NCORES):
        bs = [NB * c + i for i in range(NB)]
        m = dict(shared)
        m["xkT"] = np.ascontiguousarray(
            np.stack([xk[:, b].T.reshape(2, 128, 128) for b in bs])
        )
        m["xtc"] = np.ascontiguousarray(
            np.stack([temporal[tq[:, b]].reshape(S, 2, 128).transpose(2, 0, 1) for b in bs])
        )
        m["rows"] = np.ascontiguousarray(
            np.stack([
                np.stack([-m1[:, :, b].reshape(LQ), s1[:, :, b].reshape(LQ)])
                for b in bs
            ]).astype(F32)
        )
        m["r1b"] = np.ascontiguousarray(
            np.stack([
                np.broadcast_to(r1o8[:, :, b].reshape(1, LQ), (128, LQ)) for b in bs
            ])
        )
        m["klog"] = np.ascontiguousarray(np.stack([klog[b] for b in bs]))
        in_maps.append(m)

    nc = _get_program(vq_any, bv_any)
    from concourse.bass_utils import run_bass_kernel_spmd

    res = run_bass_kernel_spmd(nc, in_maps, core_ids=list(range(NCORES)))
    global _LAST_RES
    _LAST_RES = res

    tx = np.zeros((S, B, 2, 64, 64), F32)
    hx = np.zeros((S, B, 2, 24, 24), F32)
    lg = np.zeros((S, B, 100), F32)
    for c in range(NCORES):
        r = res.results[c]
        txs = r["txs"].reshape(NB, S, 4, 4, 2, 16, 16).transpose(1, 0, 4, 5, 2, 6, 3)
        tx[:, NB * c : NB * c + NB] = txs.reshape(S, NB, 2, 64, 64)
        hxs = r["hxs"].reshape(NB, S, 4, 4, 2, 6, 6).transpose(1, 0, 4, 5, 2, 6, 3)
        hx[:, NB * c : NB * c + NB] = hxs.reshape(S, NB, 2, 24, 24)
        lg[:, NB * c : NB * c + NB] = r["lgs"].transpose(1, 0, 2)

    return lg[..., :20], lg[..., 20:84], lg[..., 84:], tx, hx
